# revision 4
# baseline (speedup 1.0000x reference)
"""Trainium2 Bass kernel for nn_DehazeBlock:
dilated 3x3 conv (d=2, same-pad) -> BatchNorm2d (training-mode, batch stats)
-> ReLU -> per-4x4-block spatial sort; output concat([a, sorted(a)], channel).

Sharding: data-parallel over batch (16 images -> 8 cores x 2 images).
BN batch stats are all-reduced across the 8 cores inside the kernel.

Key algebraic facts used:
 - conv bias is absorbed by the BN mean subtraction -> ignored entirely.
 - relu(scale*v + bias) with scale > 0 is monotone nondecreasing, so
   sort(relu(affine(x))) == relu(affine(sort(x))).  The 4x4 block sort
   therefore runs on the RAW conv output in phase 1 (overlapped with the
   conv itself), and the BN affine + ReLU is applied afterwards in
   phase 2, once the cross-core stats collective has completed.

Pipeline per core:
  phase 1 (per 32-row chunk of each of the 2 images, processed together
           on partition halves 0-63 / 64-127):
    - DMA a padded input band to SBUF (bf16; host pre-converts).
    - 9 taps x 2 images of K=64 matmuls accumulate the conv in PSUM,
      img0 in PE quadrant (0,0) and img1 in (64,64) so they run
      concurrently in the 128x128 array.
    - ScalarE copies PSUM->SBUF (bf16, block-lane-major layout) with
      accum_out producing the per-channel sum for BN.
    - VectorE squares (scalar_tensor_tensor) for the per-channel sumsq.
    - VectorE runs a 63-comparator Batcher odd-even mergesort network
      over the 16 block lanes (min/max tensor_tensor at bf16 2x mode).
    - conv chunk and sorted chunk are spilled to DRAM.
  collective: 64x2 fp32 (sum, sumsq) AllReduce over the 8 cores;
    scale = gamma*rsqrt(var+eps), bias = beta - mean*scale on-chip.
  phase 2: reload both streams, ScalarE applies relu(scale*x+bias)
    while un-permuting lane-major -> natural layout, DMA out fp32.
"""

import sys

import numpy as np
import ml_dtypes

for _p in ("/opt/trn_rl_repo",):
    if _p not in sys.path:
        sys.path.insert(0, _p)

import concourse.bacc as bacc
import concourse.mybir as mybir
import concourse.tile as tile
from concourse.bass_utils import run_bass_kernel_spmd

NCORES = 8
NB = 2                 # images per core
C = 64                 # channels
H = W = 256
RS = 4                 # block (ranking) size
HB = H // RS           # 64 block rows
S = 8                  # block rows per chunk
NCHUNK = HB // S       # 8 chunks
LANES = RS * RS        # 16 positions within a 4x4 block
BLK = S * (W // RS)    # 512 blocks per chunk
CHW = LANES * BLK      # 8192 elements per chunk per partition
WP = 264               # band width allocated (260 used: 2+256+2 pad)
BR = 40                # band rows allocated (36 used)
NTOT = NCORES * NB * H * W   # global BN count (full batch)
EPS = 1e-5

F32 = mybir.dt.float32
BF16 = mybir.dt.bfloat16
AF = mybir.ActivationFunctionType
ALU = mybir.AluOpType


def _batcher_pairs(n=16):
    pairs = []

    def merge(lo, n2, r):
        step = r * 2
        if step < n2:
            merge(lo, n2, step)
            merge(lo + r, n2, step)
            for i in range(lo + r, lo + n2 - r, step):
                pairs.append((i, i + r))
        else:
            pairs.append((lo, lo + r))

    def sort_range(lo, hi):
        if hi - lo >= 1:
            mid = lo + (hi - lo) // 2
            sort_range(lo, mid)
            sort_range(mid + 1, hi)
            merge(lo, hi - lo + 1, 1)

    sort_range(0, n - 1)
    return pairs


def _plan_sort(pairs):
    """Register-renamed compare-exchange plan.

    Logical lanes start in the (read-only) conv chunk buffer; every CE
    writes min/max to two fresh scratch slots.  Returns (steps, final,
    nslot): steps = (src_u, src_v, dst_u, dst_v) with src ('c'|'s', idx),
    final[rank] = ('s', slot) for ranks 0..15 ascending.
    """
    pos = {l: ("c", l) for l in range(LANES)}
    free = []
    nslot = 0
    steps = []
    for (u, v) in pairs:
        su, sv = pos[u], pos[v]
        dst = []
        for _ in range(2):
            if free:
                dst.append(free.pop())
            else:
                dst.append(nslot)
                nslot += 1
        du, dv = dst
        steps.append((su, sv, du, dv))
        for old in (su, sv):
            if old[0] == "s":
                free.append(old[1])
        pos[u], pos[v] = ("s", du), ("s", dv)
    return steps, pos, nslot


_PAIRS = _batcher_pairs(LANES)
_SORT_STEPS, _SORT_FINAL, _NSLOT = _plan_sort(_PAIRS)


def _body(tc, nc, x, wT, gamma, beta, out):
    with tc.tile_pool(name="dram", bufs=1, space="DRAM") as dpool, \
         tc.tile_pool(name="persist", bufs=1) as pp:
        conv_d = dpool.tile([128, NCHUNK * CHW], BF16)
        sort_d = dpool.tile([128, NCHUNK * CHW], BF16)
        cc_in = dpool.tile([C, 2], F32)
        cc_out = dpool.tile([C, 2], F32, addr_space="Shared")

        stats_sum = pp.tile([128, NCHUNK * LANES], F32)
        stats_sq = pp.tile([128, NCHUNK * RS], F32)
        wsb = pp.tile([128, 9 * C], BF16)
        scale128 = pp.tile([128, 1], F32)
        bias128 = pp.tile([128, 1], F32)

        nc.sync.dma_start(wsb[0:C, :], wT)
        nc.sync.dma_start(wsb[C:2 * C, :], wT)
        wsb3 = wsb.rearrange("p (t co) -> p t co", co=C)

        # ---------------- phase 1: conv + stats + block sort ----------------
        with tc.tile_pool(name="band", bufs=3) as band_pool, \
             tc.tile_pool(name="psum", bufs=6, space="PSUM") as psum_pool, \
             tc.tile_pool(name="cbuf", bufs=2) as cbuf_pool, \
             tc.tile_pool(name="sortb", bufs=2) as sort_pool, \
             tc.tile_pool(name="sq", bufs=2) as sq_pool:
            for ch in range(NCHUNK):
                band = band_pool.tile([128, BR * WP], BF16)
                band3 = band.rearrange("p (r q) -> p r q", q=WP)
                r0 = RS * S * ch - 2          # x row of band row 0
                t0 = 2 if ch == 0 else 0      # valid band rows [t0, t1)
                t1 = 34 if ch == NCHUNK - 1 else 36
                if ch == 0:
                    nc.gpsimd.memset(band3[:, 0:2, 0:260], 0.0)
                if ch == NCHUNK - 1:
                    nc.gpsimd.memset(band3[:, 34:36, 0:260], 0.0)
                nc.gpsimd.memset(band3[:, 0:36, 0:2], 0.0)
                nc.gpsimd.memset(band3[:, 0:36, 258:260], 0.0)
                for m in range(NB):
                    nc.sync.dma_start(
                        band3[C * m:C * (m + 1), t0:t1, 2:258],
                        x[m, :, r0 + t0:r0 + t1, :],
                    )

                cbuf = cbuf_pool.tile([128, CHW], BF16)
                for l in range(LANES):
                    i, j = divmod(l, RS)
                    ps = psum_pool.tile([128, BLK], F32, name="ps")
                    for t in range(9):
                        ky, kx = divmod(t, 3)
                        a, b = i + 2 * ky, j + 2 * kx
                        for m in range(NB):
                            ph = slice(C * m, C * (m + 1))
                            rhs = band3[ph, a:a + 32, b:b + 256].rearrange(
                                "p (hb f) (wb g) -> p hb f wb g", f=RS, g=RS
                            )[:, :, 0, :, 0]
                            nc.tensor.matmul(
                                ps[ph, :], wsb3[ph, t, :], rhs,
                                start=(t == 0), stop=(t == 8),
                            )
                    col = ch * LANES + l
                    nc.scalar.activation(
                        cbuf[:, l * BLK:(l + 1) * BLK], ps[:, :], AF.Copy,
                        accum_out=stats_sum[:, col:col + 1],
                    )
                    if l % RS == RS - 1:
                        ig = l // RS
                        seg = cbuf[:, ig * RS * BLK:(ig + 1) * RS * BLK]
                        sq = sq_pool.tile([128, RS * BLK], BF16, name="sq")
                        nc.vector.scalar_tensor_tensor(
                            sq, seg, 0.0, seg, op0=ALU.bypass, op1=ALU.mult,
                            accum_out=stats_sq[:, ch * RS + ig:ch * RS + ig + 1],
                        )

                nc.sync.dma_start(conv_d[:, ch * CHW:(ch + 1) * CHW], cbuf)

                st = sort_pool.tile([128, _NSLOT * BLK], BF16, name="st")

                def ap_of(p):
                    kind, idx = p
                    buf = cbuf if kind == "c" else st
                    return buf[:, idx * BLK:(idx + 1) * BLK]

                for (su, sv, du, dv) in _SORT_STEPS:
                    nc.vector.tensor_tensor(
                        ap_of(("s", du)), ap_of(su), ap_of(sv), op=ALU.min)
                    nc.vector.tensor_tensor(
                        ap_of(("s", dv)), ap_of(su), ap_of(sv), op=ALU.max)
                for r in range(LANES):
                    nc.sync.dma_start(
                        sort_d[:, ch * CHW + r * BLK:ch * CHW + (r + 1) * BLK],
                        ap_of(_SORT_FINAL[r]),
                    )

        # ------------- stats finalize + cross-core all-reduce -------------
        with tc.tile_pool(name="fin", bufs=1) as fp:
            ssum = fp.tile([128, 2], F32)
            nc.vector.reduce_sum(ssum[:, 0:1], stats_sum, axis=mybir.AxisListType.X)
            nc.vector.reduce_sum(ssum[:, 1:2], stats_sq, axis=mybir.AxisListType.X)
            tmp = fp.tile([C, 2], F32)
            nc.sync.dma_start(tmp, ssum[C:2 * C, :])
            comb = fp.tile([C, 2], F32)
            nc.vector.tensor_add(comb, ssum[0:C, :], tmp)
            nc.sync.dma_start(cc_in, comb)
            nc.gpsimd.collective_compute(
                "AllReduce", ALU.add,
                replica_groups=[list(range(NCORES))],
                ins=[cc_in.opt()], outs=[cc_out.opt()],
            )
            gst = fp.tile([C, 2], F32)
            nc.sync.dma_start(gst, cc_out)

            gam = fp.tile([C, 1], F32)
            bet = fp.tile([C, 1], F32)
            nc.sync.dma_start(gam, gamma)
            nc.sync.dma_start(bet, beta)
            mean = fp.tile([C, 1], F32)
            nc.vector.tensor_scalar_mul(mean, gst[:, 0:1], 1.0 / NTOT)
            ex2 = fp.tile([C, 1], F32)
            nc.vector.tensor_scalar_mul(ex2, gst[:, 1:2], 1.0 / NTOT)
            msq = fp.tile([C, 1], F32)
            nc.vector.tensor_mul(msq, mean, mean)
            var = fp.tile([C, 1], F32)
            nc.vector.tensor_sub(var, ex2, msq)
            vpe = fp.tile([C, 1], F32)
            nc.vector.tensor_scalar_add(vpe, var, EPS)
            std = fp.tile([C, 1], F32)
            nc.scalar.sqrt(std, vpe)
            rstd = fp.tile([C, 1], F32)
            nc.vector.reciprocal(rstd, std)
            sc64 = fp.tile([C, 1], F32)
            nc.vector.tensor_mul(sc64, gam, rstd)
            t1v = fp.tile([C, 1], F32)
            nc.vector.tensor_mul(t1v, mean, sc64)
            nb64 = fp.tile([C, 1], F32)
            nc.vector.tensor_sub(nb64, bet, t1v)
            nc.sync.dma_start(scale128[0:C, :], sc64)
            nc.sync.dma_start(scale128[C:2 * C, :], sc64)
            nc.sync.dma_start(bias128[0:C, :], nb64)
            nc.sync.dma_start(bias128[C:2 * C, :], nb64)

        # ------------- phase 2: affine+relu, unpermute, write out -------------
        with tc.tile_pool(name="p2i", bufs=3) as p2i, \
             tc.tile_pool(name="p2o", bufs=3) as p2o:
            for ch in range(NCHUNK):
                for (src_d, c_off) in ((conv_d, 0), (sort_d, C)):
                    pin = p2i.tile([128, CHW], BF16, name="pin")
                    nc.sync.dma_start(pin, src_d[:, ch * CHW:(ch + 1) * CHW])
                    pout = p2o.tile([128, CHW], F32, name="pout")
                    pout5 = pout.rearrange(
                        "p (hb ii wb jj) -> p hb ii wb jj",
                        hb=S, ii=RS, wb=W // RS, jj=RS)
                    for i in range(RS):
                        dst = pout5[:, :, i, :, :].transpose([0, 3, 1, 2])
                        src = pin[:, i * RS * BLK:(i + 1) * RS * BLK]
                        if c_off == 0:
                            # a-stream: relu(scale*x+bias) fused on ScalarE
                            nc.scalar.activation(
                                dst, src, AF.Relu,
                                bias=bias128[:, 0:1], scale=scale128[:, 0:1],
                            )
                        else:
                            # y-stream on the otherwise-idle GpSimd engine
                            nc.gpsimd.tensor_scalar(
                                dst, src, scale128[:, 0:1], bias128[:, 0:1],
                                op0=ALU.mult, op1=ALU.add,
                            )
                            nc.gpsimd.tensor_scalar_max(dst, dst, 0.0)
                    for m in range(NB):
                        nc.sync.dma_start(
                            out[m, c_off:c_off + C,
                                RS * S * ch:RS * S * (ch + 1), :],
                            pout[C * m:C * (m + 1), :].rearrange(
                                "p (r q) -> p r q", q=W),
                        )


_PROGRAM = None


def _get_program():
    global _PROGRAM
    if _PROGRAM is not None:
        return _PROGRAM
    nc = bacc.Bacc("TRN2", debug=False, enable_asserts=False,
                   target_bir_lowering=False, num_devices=NCORES)
    x = nc.dram_tensor("x", [NB, C, H, W], BF16, kind="ExternalInput").ap()
    wT = nc.dram_tensor("wT", [C, 9 * C], BF16, kind="ExternalInput").ap()
    gamma = nc.dram_tensor("gamma", [C, 1], F32, kind="ExternalInput").ap()
    beta = nc.dram_tensor("beta", [C, 1], F32, kind="ExternalInput").ap()
    out = nc.dram_tensor("out", [NB, 2 * C, H, W], F32, kind="ExternalOutput").ap()
    with tile.TileContext(nc) as tc:
        _body(tc, nc, x, wT, gamma, beta, out)
    nc.compile()
    _PROGRAM = nc
    return nc


def _in_maps(x, conv_w, gamma, beta):
    bf = ml_dtypes.bfloat16
    xq = np.ascontiguousarray(np.asarray(x, np.float32)).astype(bf)
    wTm = np.ascontiguousarray(
        np.asarray(conv_w, np.float32).transpose(1, 2, 3, 0)
    ).reshape(C, 9 * C).astype(bf)
    g = np.ascontiguousarray(np.asarray(gamma, np.float32).reshape(C, 1))
    b = np.ascontiguousarray(np.asarray(beta, np.float32).reshape(C, 1))
    return [
        {"x": xq[NB * k:NB * (k + 1)], "wT": wTm, "gamma": g, "beta": b}
        for k in range(NCORES)
    ]


def run(x, conv_w, conv_b, gamma, beta, **spmd_kwargs):
    nc = _get_program()
    res = run_bass_kernel_spmd(
        nc, _in_maps(x, conv_w, gamma, beta),
        core_ids=list(range(NCORES)), **spmd_kwargs)
    full = np.concatenate(
        [res.results[k]["out"] for k in range(NCORES)], axis=0)
    return full, res


def kernel(x, conv_w, conv_b, gamma, beta):
    full, _ = run(x, conv_w, conv_b, gamma, beta)
    return full


# revision 6
# speedup vs baseline: 1.9267x; 1.9267x over previous
"""Trainium2 Bass kernel for nn_DehazeBlock:
dilated 3x3 conv (d=2, same-pad) -> BatchNorm2d (training-mode, batch stats)
-> ReLU -> per-4x4-block spatial sort; output concat([a, sorted(a)], channel).

Sharding: data-parallel over batch (16 images -> 8 cores x 2 images).
BN batch stats are all-reduced across the 8 cores inside the kernel.

Key algebraic facts used:
 - conv bias is absorbed by the BN mean subtraction -> ignored entirely.
 - relu(scale*v + bias) with scale > 0 is monotone nondecreasing, so
   sort(relu(affine(x))) == relu(affine(sort(x))).  The 4x4 block sort
   therefore runs on the RAW conv output in phase 1 (overlapped with the
   conv itself), and the BN affine + ReLU is applied afterwards in
   phase 2, once the cross-core stats collective has completed.

Pipeline per core:
  phase 1 (per 32-row chunk of each of the 2 images, processed together
           on partition halves 0-63 / 64-127):
    - DMA a padded input band to SBUF (bf16; host pre-converts).
    - 9 taps x 2 images of K=64 matmuls accumulate the conv in PSUM,
      img0 in PE quadrant (0,0) and img1 in (64,64) so they run
      concurrently in the 128x128 array.
    - ScalarE copies PSUM->SBUF (bf16, block-lane-major layout) with
      accum_out producing the per-channel sum for BN.
    - VectorE squares (scalar_tensor_tensor) for the per-channel sumsq.
    - VectorE runs a 63-comparator Batcher odd-even mergesort network
      over the 16 block lanes (min/max tensor_tensor at bf16 2x mode).
    - conv chunk and sorted chunk are spilled to DRAM.
  collective: 64x2 fp32 (sum, sumsq) AllReduce over the 8 cores;
    scale = gamma*rsqrt(var+eps), bias = beta - mean*scale on-chip.
  phase 2: reload both streams, ScalarE applies relu(scale*x+bias)
    while un-permuting lane-major -> natural layout, DMA out fp32.
"""

import sys

import numpy as np
import ml_dtypes

for _p in ("/opt/trn_rl_repo",):
    if _p not in sys.path:
        sys.path.insert(0, _p)

import concourse.bacc as bacc
import concourse.mybir as mybir
import concourse.tile as tile
from concourse.bass_utils import run_bass_kernel_spmd

NCORES = 8
NB = 2                 # images per core
C = 64                 # channels
H = W = 256
RS = 4                 # block (ranking) size
HB = H // RS           # 64 block rows
S = 8                  # block rows per chunk
NCHUNK = HB // S       # 8 chunks
LANES = RS * RS        # 16 positions within a 4x4 block
BLK = S * (W // RS)    # 512 blocks per chunk
CHW = LANES * BLK      # 8192 elements per chunk per partition
WP = 264               # band width allocated (260 used: 2+256+2 pad)
BR = 40                # band rows allocated (36 used)
NTOT = NCORES * NB * H * W   # global BN count (full batch)
EPS = 1e-5

F32 = mybir.dt.float32
BF16 = mybir.dt.bfloat16
AF = mybir.ActivationFunctionType
ALU = mybir.AluOpType


def _batcher_pairs(n=16):
    pairs = []

    def merge(lo, n2, r):
        step = r * 2
        if step < n2:
            merge(lo, n2, step)
            merge(lo + r, n2, step)
            for i in range(lo + r, lo + n2 - r, step):
                pairs.append((i, i + r))
        else:
            pairs.append((lo, lo + r))

    def sort_range(lo, hi):
        if hi - lo >= 1:
            mid = lo + (hi - lo) // 2
            sort_range(lo, mid)
            sort_range(mid + 1, hi)
            merge(lo, hi - lo + 1, 1)

    sort_range(0, n - 1)
    return pairs


def _plan_sort(pairs):
    """Register-renamed compare-exchange plan.

    Logical lanes start in the (read-only) conv chunk buffer; every CE
    writes min/max to two fresh scratch slots.  Returns (steps, final,
    nslot): steps = (src_u, src_v, dst_u, dst_v) with src ('c'|'s', idx),
    final[rank] = ('s', slot) for ranks 0..15 ascending.
    """
    pos = {l: ("c", l) for l in range(LANES)}
    free = []
    nslot = 0
    steps = []
    for (u, v) in pairs:
        su, sv = pos[u], pos[v]
        dst = []
        for _ in range(2):
            if free:
                dst.append(free.pop())
            else:
                dst.append(nslot)
                nslot += 1
        du, dv = dst
        steps.append((su, sv, du, dv))
        for old in (su, sv):
            if old[0] == "s":
                free.append(old[1])
        pos[u], pos[v] = ("s", du), ("s", dv)
    return steps, pos, nslot


_PAIRS = _batcher_pairs(LANES)
_SORT_STEPS, _SORT_FINAL, _NSLOT = _plan_sort(_PAIRS)


def _body(tc, nc, x, wT, gamma, beta, out):
    with tc.tile_pool(name="dram", bufs=1, space="DRAM") as dpool, \
         tc.tile_pool(name="persist", bufs=1) as pp:
        conv_d = dpool.tile([128, NCHUNK * CHW], BF16)
        sort_d = dpool.tile([128, NCHUNK * CHW], BF16)
        cc_in = dpool.tile([C, 2], F32)
        cc_out = dpool.tile([C, 2], F32, addr_space="Shared")

        stats_sum = pp.tile([128, NCHUNK * LANES], F32)
        stats_sq = pp.tile([128, NCHUNK * RS], F32)
        wsb = pp.tile([128, 9 * C], BF16)
        scale128 = pp.tile([128, 1], F32)
        bias128 = pp.tile([128, 1], F32)

        nc.sync.dma_start(wsb[0:C, :], wT)
        nc.sync.dma_start(wsb[C:2 * C, :], wT)
        wsb3 = wsb.rearrange("p (t co) -> p t co", co=C)

        # ---------------- phase 1: conv + stats + block sort ----------------
        with tc.tile_pool(name="band", bufs=3) as band_pool, \
             tc.tile_pool(name="psum", bufs=6, space="PSUM") as psum_pool, \
             tc.tile_pool(name="cbuf", bufs=2) as cbuf_pool, \
             tc.tile_pool(name="sortb", bufs=2) as sort_pool, \
             tc.tile_pool(name="sq", bufs=2) as sq_pool:
            for ch in range(NCHUNK):
                band = band_pool.tile([128, BR * WP], BF16)
                band3 = band.rearrange("p (r q) -> p r q", q=WP)
                r0 = RS * S * ch - 2          # x row of band row 0
                t0 = 2 if ch == 0 else 0      # valid band rows [t0, t1)
                t1 = 34 if ch == NCHUNK - 1 else 36
                if ch == 0:
                    nc.gpsimd.memset(band3[:, 0:2, 0:260], 0.0)
                if ch == NCHUNK - 1:
                    nc.gpsimd.memset(band3[:, 34:36, 0:260], 0.0)
                nc.gpsimd.memset(band3[:, 0:36, 0:2], 0.0)
                nc.gpsimd.memset(band3[:, 0:36, 258:260], 0.0)
                for m in range(NB):
                    nc.sync.dma_start(
                        band3[C * m:C * (m + 1), t0:t1, 2:258],
                        x[m, :, r0 + t0:r0 + t1, :],
                    )

                cbuf = cbuf_pool.tile([128, CHW], BF16)
                # lane-major view: [p, i, j, hbl, wb]
                cbuf5 = cbuf.rearrange(
                    "p (ii jj hb wb) -> p ii jj hb wb",
                    ii=RS, jj=RS, hb=S, wb=W // RS)
                # 16 tiles of 2 natural output rows; contiguous rhs for the PE
                for k in range(LANES):
                    hbl, half = divmod(k, 2)
                    i0 = 2 * half             # out rows 4*hbl + i0 (+1)
                    y0 = 4 * hbl + i0         # chunk-local out row
                    ps = psum_pool.tile([128, BLK], F32, name="ps")
                    for t in range(9):
                        ky, kx = divmod(t, 3)
                        for m in range(NB):
                            ph = slice(C * m, C * (m + 1))
                            rhs = band3[ph, y0 + 2 * ky:y0 + 2 * ky + 2,
                                        2 * kx:2 * kx + 256]
                            nc.tensor.matmul(
                                ps[ph, :], wsb3[ph, t, :], rhs,
                                start=(t == 0), stop=(t == 8),
                            )
                    col = ch * LANES + k
                    # scatter 2 natural rows into lane-major: free order
                    # (i, w=(wb, j)) -> dims (ii, wb, jj)
                    dst = cbuf5[:, i0:i0 + 2, :, hbl, :].transpose([0, 1, 3, 2])
                    nc.scalar.activation(
                        dst, ps[:, :], AF.Copy,
                        accum_out=stats_sum[:, col:col + 1],
                    )
                for ig in range(RS):
                    seg = cbuf[:, ig * RS * BLK:(ig + 1) * RS * BLK]
                    sq = sq_pool.tile([128, RS * BLK], BF16, name="sq")
                    nc.vector.scalar_tensor_tensor(
                        sq, seg, 0.0, seg, op0=ALU.bypass, op1=ALU.mult,
                        accum_out=stats_sq[:, ch * RS + ig:ch * RS + ig + 1],
                    )

                nc.sync.dma_start(conv_d[:, ch * CHW:(ch + 1) * CHW], cbuf)

                st = sort_pool.tile([128, _NSLOT * BLK], BF16, name="st")

                def ap_of(p):
                    kind, idx = p
                    buf = cbuf if kind == "c" else st
                    return buf[:, idx * BLK:(idx + 1) * BLK]

                for (su, sv, du, dv) in _SORT_STEPS:
                    nc.vector.tensor_tensor(
                        ap_of(("s", du)), ap_of(su), ap_of(sv), op=ALU.min)
                    nc.vector.tensor_tensor(
                        ap_of(("s", dv)), ap_of(su), ap_of(sv), op=ALU.max)
                for r in range(LANES):
                    nc.sync.dma_start(
                        sort_d[:, ch * CHW + r * BLK:ch * CHW + (r + 1) * BLK],
                        ap_of(_SORT_FINAL[r]),
                    )

        # ------------- stats finalize + cross-core all-reduce -------------
        with tc.tile_pool(name="fin", bufs=1) as fp:
            ssum = fp.tile([128, 2], F32)
            nc.vector.reduce_sum(ssum[:, 0:1], stats_sum, axis=mybir.AxisListType.X)
            nc.vector.reduce_sum(ssum[:, 1:2], stats_sq, axis=mybir.AxisListType.X)
            tmp = fp.tile([C, 2], F32)
            nc.sync.dma_start(tmp, ssum[C:2 * C, :])
            comb = fp.tile([C, 2], F32)
            nc.vector.tensor_add(comb, ssum[0:C, :], tmp)
            nc.sync.dma_start(cc_in, comb)
            nc.gpsimd.collective_compute(
                "AllReduce", ALU.add,
                replica_groups=[list(range(NCORES))],
                ins=[cc_in.opt()], outs=[cc_out.opt()],
            )
            gst = fp.tile([C, 2], F32)
            nc.sync.dma_start(gst, cc_out)

            gam = fp.tile([C, 1], F32)
            bet = fp.tile([C, 1], F32)
            nc.sync.dma_start(gam, gamma)
            nc.sync.dma_start(bet, beta)
            mean = fp.tile([C, 1], F32)
            nc.vector.tensor_scalar_mul(mean, gst[:, 0:1], 1.0 / NTOT)
            ex2 = fp.tile([C, 1], F32)
            nc.vector.tensor_scalar_mul(ex2, gst[:, 1:2], 1.0 / NTOT)
            msq = fp.tile([C, 1], F32)
            nc.vector.tensor_mul(msq, mean, mean)
            var = fp.tile([C, 1], F32)
            nc.vector.tensor_sub(var, ex2, msq)
            vpe = fp.tile([C, 1], F32)
            nc.vector.tensor_scalar_add(vpe, var, EPS)
            std = fp.tile([C, 1], F32)
            nc.scalar.sqrt(std, vpe)
            rstd = fp.tile([C, 1], F32)
            nc.vector.reciprocal(rstd, std)
            sc64 = fp.tile([C, 1], F32)
            nc.vector.tensor_mul(sc64, gam, rstd)
            t1v = fp.tile([C, 1], F32)
            nc.vector.tensor_mul(t1v, mean, sc64)
            nb64 = fp.tile([C, 1], F32)
            nc.vector.tensor_sub(nb64, bet, t1v)
            nc.sync.dma_start(scale128[0:C, :], sc64)
            nc.sync.dma_start(scale128[C:2 * C, :], sc64)
            nc.sync.dma_start(bias128[0:C, :], nb64)
            nc.sync.dma_start(bias128[C:2 * C, :], nb64)

        # ------------- phase 2: affine+relu, unpermute, write out -------------
        with tc.tile_pool(name="p2i", bufs=3) as p2i, \
             tc.tile_pool(name="p2o", bufs=3) as p2o:
            for ch in range(NCHUNK):
                for (src_d, c_off) in ((conv_d, 0), (sort_d, C)):
                    pin = p2i.tile([128, CHW], BF16, name="pin")
                    nc.sync.dma_start(pin, src_d[:, ch * CHW:(ch + 1) * CHW])
                    pout = p2o.tile([128, CHW], F32, name="pout")
                    pout5 = pout.rearrange(
                        "p (hb ii wb jj) -> p hb ii wb jj",
                        hb=S, ii=RS, wb=W // RS, jj=RS)
                    for i in range(RS):
                        dst = pout5[:, :, i, :, :].transpose([0, 3, 1, 2])
                        src = pin[:, i * RS * BLK:(i + 1) * RS * BLK]
                        if c_off == 0:
                            # a-stream: relu(scale*x+bias) fused on ScalarE
                            nc.scalar.activation(
                                dst, src, AF.Relu,
                                bias=bias128[:, 0:1], scale=scale128[:, 0:1],
                            )
                        else:
                            # y-stream on the otherwise-idle Vector engine
                            nc.vector.tensor_scalar(
                                dst, src, scale128[:, 0:1], bias128[:, 0:1],
                                op0=ALU.mult, op1=ALU.add,
                            )
                            nc.vector.tensor_scalar_max(dst, dst, 0.0)
                    for m in range(NB):
                        nc.sync.dma_start(
                            out[m, c_off:c_off + C,
                                RS * S * ch:RS * S * (ch + 1), :],
                            pout[C * m:C * (m + 1), :].rearrange(
                                "p (r q) -> p r q", q=W),
                        )


_PROGRAM = None


def _get_program():
    global _PROGRAM
    if _PROGRAM is not None:
        return _PROGRAM
    nc = bacc.Bacc("TRN2", debug=False, enable_asserts=False,
                   target_bir_lowering=False, num_devices=NCORES)
    x = nc.dram_tensor("x", [NB, C, H, W], BF16, kind="ExternalInput").ap()
    wT = nc.dram_tensor("wT", [C, 9 * C], BF16, kind="ExternalInput").ap()
    gamma = nc.dram_tensor("gamma", [C, 1], F32, kind="ExternalInput").ap()
    beta = nc.dram_tensor("beta", [C, 1], F32, kind="ExternalInput").ap()
    out = nc.dram_tensor("out", [NB, 2 * C, H, W], F32, kind="ExternalOutput").ap()
    with tile.TileContext(nc) as tc:
        _body(tc, nc, x, wT, gamma, beta, out)
    nc.compile()
    _PROGRAM = nc
    return nc


def _in_maps(x, conv_w, gamma, beta):
    bf = ml_dtypes.bfloat16
    xq = np.ascontiguousarray(np.asarray(x, np.float32)).astype(bf)
    wTm = np.ascontiguousarray(
        np.asarray(conv_w, np.float32).transpose(1, 2, 3, 0)
    ).reshape(C, 9 * C).astype(bf)
    g = np.ascontiguousarray(np.asarray(gamma, np.float32).reshape(C, 1))
    b = np.ascontiguousarray(np.asarray(beta, np.float32).reshape(C, 1))
    return [
        {"x": xq[NB * k:NB * (k + 1)], "wT": wTm, "gamma": g, "beta": b}
        for k in range(NCORES)
    ]


def run(x, conv_w, conv_b, gamma, beta, **spmd_kwargs):
    nc = _get_program()
    res = run_bass_kernel_spmd(
        nc, _in_maps(x, conv_w, gamma, beta),
        core_ids=list(range(NCORES)), **spmd_kwargs)
    full = np.concatenate(
        [res.results[k]["out"] for k in range(NCORES)], axis=0)
    return full, res


def kernel(x, conv_w, conv_b, gamma, beta):
    full, _ = run(x, conv_w, conv_b, gamma, beta)
    return full


# revision 10
# speedup vs baseline: 2.2595x; 1.1727x over previous
"""Trainium2 Bass kernel for nn_DehazeBlock:
dilated 3x3 conv (d=2, same-pad) -> BatchNorm2d (training-mode, batch stats)
-> ReLU -> per-4x4-block spatial sort; output concat([a, sorted(a)], channel).

Sharding: data-parallel over batch (16 images -> 8 cores x 2 images).
BN batch stats are all-reduced across the 8 cores inside the kernel.

Key algebraic facts used:
 - conv bias is absorbed by the BN mean subtraction -> ignored entirely.
 - relu(scale*v + bias) with scale > 0 is monotone nondecreasing, so
   sort(relu(affine(x))) == relu(affine(sort(x))).  The 4x4 block sort
   therefore runs on the RAW conv output in phase 1 (overlapped with the
   conv itself), and the BN affine + ReLU is applied afterwards in
   phase 2, once the cross-core stats collective has completed.

Pipeline per core:
  phase 1 (per 32-row chunk of each of the 2 images, processed together
           on partition halves 0-63 / 64-127):
    - DMA a padded input band to SBUF (bf16; host pre-converts).
    - 9 taps x 2 images of K=64 matmuls accumulate the conv in PSUM,
      img0 in PE quadrant (0,0) and img1 in (64,64) so they run
      concurrently in the 128x128 array.
    - ScalarE copies PSUM->SBUF (bf16, block-lane-major layout) with
      accum_out producing the per-channel sum for BN.
    - VectorE squares (scalar_tensor_tensor) for the per-channel sumsq.
    - VectorE runs a 63-comparator Batcher odd-even mergesort network
      over the 16 block lanes (min/max tensor_tensor at bf16 2x mode).
    - conv chunk and sorted chunk are spilled to DRAM.
  collective: 64x2 fp32 (sum, sumsq) AllReduce over the 8 cores;
    scale = gamma*rsqrt(var+eps), bias = beta - mean*scale on-chip.
  phase 2: reload both streams, ScalarE applies relu(scale*x+bias)
    while un-permuting lane-major -> natural layout, DMA out fp32.
"""

import sys

import numpy as np
import ml_dtypes

for _p in ("/opt/trn_rl_repo",):
    if _p not in sys.path:
        sys.path.insert(0, _p)

import concourse.bacc as bacc
import concourse.mybir as mybir
import concourse.tile as tile
from concourse.bass_utils import run_bass_kernel_spmd

NCORES = 8
NB = 2                 # images per core
C = 64                 # channels
H = W = 256
RS = 4                 # block (ranking) size
HB = H // RS           # 64 block rows
S = 8                  # block rows per chunk
NCHUNK = HB // S       # 8 chunks
LANES = RS * RS        # 16 positions within a 4x4 block
BLK = S * (W // RS)    # 512 blocks per chunk
CHW = LANES * BLK      # 8192 elements per chunk per partition
WP = 264               # band width allocated (260 used: 2+256+2 pad)
BR = 40                # band rows allocated (36 used)
NTOT = NCORES * NB * H * W   # global BN count (full batch)
EPS = 1e-5

F32 = mybir.dt.float32
BF16 = mybir.dt.bfloat16
AF = mybir.ActivationFunctionType
ALU = mybir.AluOpType


def _batcher_pairs(n=16):
    pairs = []

    def merge(lo, n2, r):
        step = r * 2
        if step < n2:
            merge(lo, n2, step)
            merge(lo + r, n2, step)
            for i in range(lo + r, lo + n2 - r, step):
                pairs.append((i, i + r))
        else:
            pairs.append((lo, lo + r))

    def sort_range(lo, hi):
        if hi - lo >= 1:
            mid = lo + (hi - lo) // 2
            sort_range(lo, mid)
            sort_range(mid + 1, hi)
            merge(lo, hi - lo + 1, 1)

    sort_range(0, n - 1)
    return pairs


def _plan_sort(pairs):
    """Register-renamed compare-exchange plan.

    Logical lanes start in the (read-only) conv chunk buffer; every CE
    writes min/max to two fresh scratch slots.  Returns (steps, final,
    nslot): steps = (src_u, src_v, dst_u, dst_v) with src ('c'|'s', idx),
    final[rank] = ('s', slot) for ranks 0..15 ascending.
    """
    pos = {l: ("c", l) for l in range(LANES)}
    free = []
    nslot = 0
    steps = []
    for (u, v) in pairs:
        su, sv = pos[u], pos[v]
        dst = []
        for _ in range(2):
            if free:
                dst.append(free.pop())
            else:
                dst.append(nslot)
                nslot += 1
        du, dv = dst
        steps.append((su, sv, du, dv))
        for old in (su, sv):
            if old[0] == "s":
                free.append(old[1])
        pos[u], pos[v] = ("s", du), ("s", dv)
    return steps, pos, nslot


_PAIRS = _batcher_pairs(LANES)
_SORT_STEPS, _SORT_FINAL, _NSLOT = _plan_sort(_PAIRS)


def _body(tc, nc, x, wT, gamma, beta, out):
    with tc.tile_pool(name="dram", bufs=1, space="DRAM") as dpool, \
         tc.tile_pool(name="persist", bufs=1) as pp:
        conv_d = dpool.tile([128, NCHUNK * CHW], BF16)
        sort_d = dpool.tile([128, NCHUNK * CHW], BF16)
        cc_in = dpool.tile([C, 2], F32)
        cc_out = dpool.tile([C, 2], F32, addr_space="Shared")

        stats_sum = pp.tile([128, NCHUNK * LANES], F32)
        stats_sq = pp.tile([128, NCHUNK * RS], F32)
        wsb = pp.tile([128, 9 * C], BF16)
        scale128 = pp.tile([128, 1], F32)
        bias128 = pp.tile([128, 1], F32)

        nc.sync.dma_start(wsb[0:C, :], wT)
        nc.sync.dma_start(wsb[C:2 * C, :], wT)
        wsb3 = wsb.rearrange("p (t co) -> p t co", co=C)

        # ---------------- phase 1: conv + stats + block sort ----------------
        # chunks are processed in PAIRS: both halves land in one pair buffer,
        # the sort then runs at FD=1024 (2x fewer DVE ops vs per-chunk).
        with tc.tile_pool(name="band", bufs=3) as band_pool, \
             tc.tile_pool(name="psum", bufs=6, space="PSUM") as psum_pool, \
             tc.tile_pool(name="cbuf", bufs=2) as cbuf_pool, \
             tc.tile_pool(name="sortb", bufs=1) as sort_pool, \
             tc.tile_pool(name="sq", bufs=2) as sq_pool, \
             tc.tile_pool(name="fin", bufs=1) as fp:
            sort_d8 = sort_d.rearrange("p (cc lf) -> p cc lf", cc=NCHUNK)
            pbuf = None
            for ch in range(NCHUNK):
                q = ch % 2
                band = band_pool.tile([128, BR * WP], BF16)
                band3 = band.rearrange("p (r w) -> p r w", w=WP)
                r0 = RS * S * ch - 2          # x row of band row 0
                t0 = 2 if ch == 0 else 0      # valid band rows [t0, t1)
                t1 = 34 if ch == NCHUNK - 1 else 36
                if ch == 0:
                    nc.gpsimd.memset(band3[:, 0:2, 0:260], 0.0)
                if ch == NCHUNK - 1:
                    nc.gpsimd.memset(band3[:, 34:36, 0:260], 0.0)
                nc.gpsimd.memset(band3[:, 0:36, 0:2], 0.0)
                nc.gpsimd.memset(band3[:, 0:36, 258:260], 0.0)
                for m in range(NB):
                    nc.sync.dma_start(
                        band3[C * m:C * (m + 1), t0:t1, 2:258],
                        x[m, :, r0 + t0:r0 + t1, :],
                    )

                if q == 0:
                    pbuf = cbuf_pool.tile([128, 2 * CHW], BF16, name="pbuf")
                # lane-major view: [p, q, i, j, hbl, wb]
                pbuf6 = pbuf.rearrange(
                    "p (qq ii jj hb wb) -> p qq ii jj hb wb",
                    qq=2, ii=RS, jj=RS, hb=S, wb=W // RS)
                # 16 tiles of 2 natural output rows; contiguous rhs for the PE
                for k in range(LANES):
                    hbl, half = divmod(k, 2)
                    i0 = 2 * half             # out rows 4*hbl + i0 (+1)
                    y0 = 4 * hbl + i0         # chunk-local out row
                    ps = psum_pool.tile([128, BLK], F32, name="ps")
                    for t in range(9):
                        ky, kx = divmod(t, 3)
                        for m in range(NB):
                            ph = slice(C * m, C * (m + 1))
                            rhs = band3[ph, y0 + 2 * ky:y0 + 2 * ky + 2,
                                        2 * kx:2 * kx + 256]
                            nc.tensor.matmul(
                                ps[ph, :], wsb3[ph, t, :], rhs,
                                start=(t == 0), stop=(t == 8),
                            )
                    col = ch * LANES + k
                    # scatter 2 natural rows into lane-major: free order
                    # (i, w=(wb, j)) -> dims (ii, wb, jj)
                    dst = pbuf6[:, q, i0:i0 + 2, :, hbl, :].transpose([0, 1, 3, 2])
                    nc.scalar.activation(
                        dst, ps[:, :], AF.Copy,
                        accum_out=stats_sum[:, col:col + 1],
                    )
                for ig in range(RS):
                    seg = pbuf[:, q * CHW + ig * RS * BLK:
                               q * CHW + (ig + 1) * RS * BLK]
                    sq = sq_pool.tile([128, RS * BLK], BF16, name="sq")
                    nc.vector.scalar_tensor_tensor(
                        sq, seg, 0.0, seg, op0=ALU.bypass, op1=ALU.mult,
                        accum_out=stats_sq[:, ch * RS + ig:ch * RS + ig + 1],
                    )

                nc.gpsimd.dma_start(
                    conv_d[:, ch * CHW:(ch + 1) * CHW],
                    pbuf[:, q * CHW:(q + 1) * CHW])

                if ch == NCHUNK - 1:
                    # emit the stats+collective BEFORE the last sort so the
                    # DVE FIFO doesn't delay the AllReduce behind ~60us of
                    # comparators; phase 2 can then start while it drains.
                    _emit_stats(nc, fp, stats_sum, stats_sq, cc_in, cc_out,
                                gamma, beta, scale128, bias128)

                if q == 1:
                    st = sort_pool.tile([128, _NSLOT * 2 * BLK], BF16, name="st")
                    pbufq = pbuf.rearrange("p (qq lf) -> p qq lf", qq=2)

                    def ap_of(pos):
                        kind, idx = pos
                        if kind == "c":
                            return pbufq[:, :, idx * BLK:(idx + 1) * BLK]
                        return st[:, idx * 2 * BLK:(idx + 1) * 2 * BLK] \
                            .rearrange("p (qq f) -> p qq f", qq=2)

                    for (su, sv, du, dv) in _SORT_STEPS:
                        nc.vector.tensor_tensor(
                            ap_of(("s", du)), ap_of(su), ap_of(sv), op=ALU.min)
                        nc.vector.tensor_tensor(
                            ap_of(("s", dv)), ap_of(su), ap_of(sv), op=ALU.max)
                    for r in range(LANES):
                        nc.gpsimd.dma_start(
                            sort_d8[:, ch - 1:ch + 1, r * BLK:(r + 1) * BLK],
                            ap_of(_SORT_FINAL[r]),
                        )


        _emit_phase2(tc, nc, conv_d, sort_d, out, scale128, bias128)


def _emit_stats(nc, fp, stats_sum, stats_sq, cc_in, cc_out,
                gamma, beta, scale128, bias128):
        if True:
            ssum = fp.tile([128, 2], F32)
            nc.vector.reduce_sum(ssum[:, 0:1], stats_sum, axis=mybir.AxisListType.X)
            nc.vector.reduce_sum(ssum[:, 1:2], stats_sq, axis=mybir.AxisListType.X)
            tmp = fp.tile([C, 2], F32)
            nc.sync.dma_start(tmp, ssum[C:2 * C, :])
            comb = fp.tile([C, 2], F32)
            nc.vector.tensor_add(comb, ssum[0:C, :], tmp)
            nc.sync.dma_start(cc_in, comb)
            nc.gpsimd.collective_compute(
                "AllReduce", ALU.add,
                replica_groups=[list(range(NCORES))],
                ins=[cc_in.opt()], outs=[cc_out.opt()],
            )
            gst = fp.tile([C, 2], F32)
            nc.sync.dma_start(gst, cc_out)

            gam = fp.tile([C, 1], F32)
            bet = fp.tile([C, 1], F32)
            nc.sync.dma_start(gam, gamma)
            nc.sync.dma_start(bet, beta)
            mean = fp.tile([C, 1], F32)
            nc.vector.tensor_scalar_mul(mean, gst[:, 0:1], 1.0 / NTOT)
            ex2 = fp.tile([C, 1], F32)
            nc.vector.tensor_scalar_mul(ex2, gst[:, 1:2], 1.0 / NTOT)
            msq = fp.tile([C, 1], F32)
            nc.vector.tensor_mul(msq, mean, mean)
            var = fp.tile([C, 1], F32)
            nc.vector.tensor_sub(var, ex2, msq)
            vpe = fp.tile([C, 1], F32)
            nc.vector.tensor_scalar_add(vpe, var, EPS)
            std = fp.tile([C, 1], F32)
            nc.scalar.sqrt(std, vpe)
            rstd = fp.tile([C, 1], F32)
            nc.vector.reciprocal(rstd, std)
            sc64 = fp.tile([C, 1], F32)
            nc.vector.tensor_mul(sc64, gam, rstd)
            t1v = fp.tile([C, 1], F32)
            nc.vector.tensor_mul(t1v, mean, sc64)
            nb64 = fp.tile([C, 1], F32)
            nc.vector.tensor_sub(nb64, bet, t1v)
            nc.sync.dma_start(scale128[0:C, :], sc64)
            nc.sync.dma_start(scale128[C:2 * C, :], sc64)
            nc.sync.dma_start(bias128[0:C, :], nb64)
            nc.sync.dma_start(bias128[C:2 * C, :], nb64)

def _emit_phase2(tc, nc, conv_d, sort_d, out, scale128, bias128):
        # ------------- phase 2: affine+relu, unpermute, write out -------------
        with tc.tile_pool(name="p2i", bufs=3) as p2i, \
             tc.tile_pool(name="p2o", bufs=3) as p2o:
            for ch in range(NCHUNK):
                for (src_d, c_off) in ((conv_d, 0), (sort_d, C)):
                    pin = p2i.tile([128, CHW], BF16, name="pin")
                    nc.sync.dma_start(pin, src_d[:, ch * CHW:(ch + 1) * CHW])
                    pout = p2o.tile([128, CHW], F32, name="pout")
                    pout5 = pout.rearrange(
                        "p (hb ii wb jj) -> p hb ii wb jj",
                        hb=S, ii=RS, wb=W // RS, jj=RS)
                    for i in range(RS):
                        dst = pout5[:, :, i, :, :].transpose([0, 3, 1, 2])
                        src = pin[:, i * RS * BLK:(i + 1) * RS * BLK]
                        # relu(scale*x+bias) fused in one ScalarE op
                        nc.scalar.activation(
                            dst, src, AF.Relu,
                            bias=bias128[:, 0:1], scale=scale128[:, 0:1],
                        )
                    for m in range(NB):
                        nc.gpsimd.dma_start(
                            out[m, c_off:c_off + C,
                                RS * S * ch:RS * S * (ch + 1), :],
                            pout[C * m:C * (m + 1), :].rearrange(
                                "p (r q) -> p r q", q=W),
                        )


_PROGRAM = None


def _get_program():
    global _PROGRAM
    if _PROGRAM is not None:
        return _PROGRAM
    nc = bacc.Bacc("TRN2", debug=False, enable_asserts=False,
                   target_bir_lowering=False, num_devices=NCORES)
    x = nc.dram_tensor("x", [NB, C, H, W], BF16, kind="ExternalInput").ap()
    wT = nc.dram_tensor("wT", [C, 9 * C], BF16, kind="ExternalInput").ap()
    gamma = nc.dram_tensor("gamma", [C, 1], F32, kind="ExternalInput").ap()
    beta = nc.dram_tensor("beta", [C, 1], F32, kind="ExternalInput").ap()
    out = nc.dram_tensor("out", [NB, 2 * C, H, W], F32, kind="ExternalOutput").ap()
    with tile.TileContext(nc) as tc:
        _body(tc, nc, x, wT, gamma, beta, out)
    nc.compile()
    _PROGRAM = nc
    return nc


def _in_maps(x, conv_w, gamma, beta):
    bf = ml_dtypes.bfloat16
    xq = np.ascontiguousarray(np.asarray(x, np.float32)).astype(bf)
    wTm = np.ascontiguousarray(
        np.asarray(conv_w, np.float32).transpose(1, 2, 3, 0)
    ).reshape(C, 9 * C).astype(bf)
    g = np.ascontiguousarray(np.asarray(gamma, np.float32).reshape(C, 1))
    b = np.ascontiguousarray(np.asarray(beta, np.float32).reshape(C, 1))
    return [
        {"x": xq[NB * k:NB * (k + 1)], "wT": wTm, "gamma": g, "beta": b}
        for k in range(NCORES)
    ]


def run(x, conv_w, conv_b, gamma, beta, **spmd_kwargs):
    nc = _get_program()
    res = run_bass_kernel_spmd(
        nc, _in_maps(x, conv_w, gamma, beta),
        core_ids=list(range(NCORES)), **spmd_kwargs)
    full = np.concatenate(
        [res.results[k]["out"] for k in range(NCORES)], axis=0)
    return full, res


def kernel(x, conv_w, conv_b, gamma, beta):
    full, _ = run(x, conv_w, conv_b, gamma, beta)
    return full


# revision 11
# speedup vs baseline: 2.2796x; 1.0089x over previous
"""Trainium2 Bass kernel for nn_DehazeBlock:
dilated 3x3 conv (d=2, same-pad) -> BatchNorm2d (training-mode, batch stats)
-> ReLU -> per-4x4-block spatial sort; output concat([a, sorted(a)], channel).

Sharding: data-parallel over batch (16 images -> 8 cores x 2 images).
BN batch stats are all-reduced across the 8 cores inside the kernel.

Key algebraic facts used:
 - conv bias is absorbed by the BN mean subtraction -> ignored entirely.
 - relu(scale*v + bias) with scale > 0 is monotone nondecreasing, so
   sort(relu(affine(x))) == relu(affine(sort(x))).  The 4x4 block sort
   therefore runs on the RAW conv output in phase 1 (overlapped with the
   conv itself), and the BN affine + ReLU is applied afterwards in
   phase 2, once the cross-core stats collective has completed.

Pipeline per core:
  phase 1 (per 32-row chunk of each of the 2 images, processed together
           on partition halves 0-63 / 64-127):
    - DMA a padded input band to SBUF (bf16; host pre-converts).
    - 9 taps x 2 images of K=64 matmuls accumulate the conv in PSUM,
      img0 in PE quadrant (0,0) and img1 in (64,64) so they run
      concurrently in the 128x128 array.
    - ScalarE copies PSUM->SBUF (bf16, block-lane-major layout) with
      accum_out producing the per-channel sum for BN.
    - VectorE squares (scalar_tensor_tensor) for the per-channel sumsq.
    - VectorE runs a 63-comparator Batcher odd-even mergesort network
      over the 16 block lanes (min/max tensor_tensor at bf16 2x mode).
    - conv chunk and sorted chunk are spilled to DRAM.
  collective: 64x2 fp32 (sum, sumsq) AllReduce over the 8 cores;
    scale = gamma*rsqrt(var+eps), bias = beta - mean*scale on-chip.
  phase 2: reload both streams, ScalarE applies relu(scale*x+bias)
    while un-permuting lane-major -> natural layout, DMA out fp32.
"""

import sys

import numpy as np
import ml_dtypes

for _p in ("/opt/trn_rl_repo",):
    if _p not in sys.path:
        sys.path.insert(0, _p)

import concourse.bacc as bacc
import concourse.mybir as mybir
import concourse.tile as tile
from concourse.bass_utils import run_bass_kernel_spmd

NCORES = 8
NB = 2                 # images per core
C = 64                 # channels
H = W = 256
RS = 4                 # block (ranking) size
HB = H // RS           # 64 block rows
S = 8                  # block rows per chunk
NCHUNK = HB // S       # 8 chunks
LANES = RS * RS        # 16 positions within a 4x4 block
BLK = S * (W // RS)    # 512 blocks per chunk
CHW = LANES * BLK      # 8192 elements per chunk per partition
WP = 264               # band width allocated (260 used: 2+256+2 pad)
BR = 40                # band rows allocated (36 used)
NTOT = NCORES * NB * H * W   # global BN count (full batch)
EPS = 1e-5

F32 = mybir.dt.float32
BF16 = mybir.dt.bfloat16
AF = mybir.ActivationFunctionType
ALU = mybir.AluOpType


def _batcher_pairs(n=16):
    pairs = []

    def merge(lo, n2, r):
        step = r * 2
        if step < n2:
            merge(lo, n2, step)
            merge(lo + r, n2, step)
            for i in range(lo + r, lo + n2 - r, step):
                pairs.append((i, i + r))
        else:
            pairs.append((lo, lo + r))

    def sort_range(lo, hi):
        if hi - lo >= 1:
            mid = lo + (hi - lo) // 2
            sort_range(lo, mid)
            sort_range(mid + 1, hi)
            merge(lo, hi - lo + 1, 1)

    sort_range(0, n - 1)
    return pairs


def _plan_sort(pairs):
    """Register-renamed compare-exchange plan.

    Logical lanes start in the (read-only) conv chunk buffer; every CE
    writes min/max to two fresh scratch slots.  Returns (steps, final,
    nslot): steps = (src_u, src_v, dst_u, dst_v) with src ('c'|'s', idx),
    final[rank] = ('s', slot) for ranks 0..15 ascending.
    """
    pos = {l: ("c", l) for l in range(LANES)}
    free = []
    nslot = 0
    steps = []
    for (u, v) in pairs:
        su, sv = pos[u], pos[v]
        dst = []
        for _ in range(2):
            if free:
                dst.append(free.pop())
            else:
                dst.append(nslot)
                nslot += 1
        du, dv = dst
        steps.append((su, sv, du, dv))
        for old in (su, sv):
            if old[0] == "s":
                free.append(old[1])
        pos[u], pos[v] = ("s", du), ("s", dv)
    return steps, pos, nslot


_PAIRS = _batcher_pairs(LANES)
_SORT_STEPS, _SORT_FINAL, _NSLOT = _plan_sort(_PAIRS)


def _body(tc, nc, x, wT, gamma, beta, out):
    with tc.tile_pool(name="dram", bufs=1, space="DRAM") as dpool, \
         tc.tile_pool(name="persist", bufs=1) as pp:
        conv_d = dpool.tile([128, NCHUNK * CHW], BF16)
        sort_d = dpool.tile([128, NCHUNK * CHW], BF16)
        cc_in = dpool.tile([C, 2], F32)
        cc_out = dpool.tile([C, 2], F32, addr_space="Shared")

        stats_sum = pp.tile([128, NCHUNK * LANES], F32)
        stats_sq = pp.tile([128, NCHUNK * RS], F32)
        wsb = pp.tile([128, 9 * C], BF16)
        scale128 = pp.tile([128, 1], F32)
        bias128 = pp.tile([128, 1], F32)

        nc.sync.dma_start(wsb[0:C, :], wT)
        nc.sync.dma_start(wsb[C:2 * C, :], wT)
        wsb3 = wsb.rearrange("p (t co) -> p t co", co=C)

        # ---------------- phase 1: conv + stats + block sort ----------------
        # chunks are processed in PAIRS: both halves land in one pair buffer,
        # the sort then runs at FD=1024 (2x fewer DVE ops vs per-chunk).
        with tc.tile_pool(name="band", bufs=3) as band_pool, \
             tc.tile_pool(name="psum", bufs=6, space="PSUM") as psum_pool, \
             tc.tile_pool(name="cbuf", bufs=2) as cbuf_pool, \
             tc.tile_pool(name="sortb", bufs=1) as sort_pool, \
             tc.tile_pool(name="sq", bufs=2) as sq_pool, \
             tc.tile_pool(name="fin", bufs=1) as fp:
            sort_d8 = sort_d.rearrange("p (cc lf) -> p cc lf", cc=NCHUNK)
            pbuf = None
            for ch in range(NCHUNK):
                q = ch % 2
                band = band_pool.tile([128, BR * WP], BF16)
                band3 = band.rearrange("p (r w) -> p r w", w=WP)
                r0 = RS * S * ch - 2          # x row of band row 0
                t0 = 2 if ch == 0 else 0      # valid band rows [t0, t1)
                t1 = 34 if ch == NCHUNK - 1 else 36
                if ch == 0:
                    nc.gpsimd.memset(band3[:, 0:2, 0:260], 0.0)
                if ch == NCHUNK - 1:
                    nc.gpsimd.memset(band3[:, 34:36, 0:260], 0.0)
                nc.gpsimd.memset(band3[:, 0:36, 0:2], 0.0)
                nc.gpsimd.memset(band3[:, 0:36, 258:260], 0.0)
                for m in range(NB):
                    nc.sync.dma_start(
                        band3[C * m:C * (m + 1), t0:t1, 2:258],
                        x[m, :, r0 + t0:r0 + t1, :],
                    )

                if q == 0:
                    pbuf = cbuf_pool.tile([128, 2 * CHW], BF16, name="pbuf")
                # lane-major view: [p, q, i, j, hbl, wb]
                pbuf6 = pbuf.rearrange(
                    "p (qq ii jj hb wb) -> p qq ii jj hb wb",
                    qq=2, ii=RS, jj=RS, hb=S, wb=W // RS)
                # 16 tiles of 2 natural output rows; contiguous rhs for the PE
                for k in range(LANES):
                    hbl, half = divmod(k, 2)
                    i0 = 2 * half             # out rows 4*hbl + i0 (+1)
                    y0 = 4 * hbl + i0         # chunk-local out row
                    ps = psum_pool.tile([128, BLK], F32, name="ps")
                    for t in range(9):
                        ky, kx = divmod(t, 3)
                        for m in range(NB):
                            ph = slice(C * m, C * (m + 1))
                            rhs = band3[ph, y0 + 2 * ky:y0 + 2 * ky + 2,
                                        2 * kx:2 * kx + 256]
                            nc.tensor.matmul(
                                ps[ph, :], wsb3[ph, t, :], rhs,
                                start=(t == 0), stop=(t == 8),
                            )
                    col = ch * LANES + k
                    # scatter 2 natural rows into lane-major: free order
                    # (i, w=(wb, j)) -> dims (ii, wb, jj)
                    dst = pbuf6[:, q, i0:i0 + 2, :, hbl, :].transpose([0, 1, 3, 2])
                    nc.scalar.activation(
                        dst, ps[:, :], AF.Copy,
                        accum_out=stats_sum[:, col:col + 1],
                    )
                for ig in range(RS):
                    seg = pbuf[:, q * CHW + ig * RS * BLK:
                               q * CHW + (ig + 1) * RS * BLK]
                    sq = sq_pool.tile([128, RS * BLK], BF16, name="sq")
                    nc.vector.scalar_tensor_tensor(
                        sq, seg, 0.0, seg, op0=ALU.bypass, op1=ALU.mult,
                        accum_out=stats_sq[:, ch * RS + ig:ch * RS + ig + 1],
                    )

                nc.gpsimd.dma_start(
                    conv_d[:, ch * CHW:(ch + 1) * CHW],
                    pbuf[:, q * CHW:(q + 1) * CHW])

                if ch == NCHUNK - 1:
                    # emit the stats+collective BEFORE the last sort so the
                    # DVE FIFO doesn't delay the AllReduce behind ~60us of
                    # comparators; phase 2 can then start while it drains.
                    _emit_stats(nc, fp, stats_sum, stats_sq, cc_in, cc_out,
                                gamma, beta, scale128, bias128)

                if q == 1:
                    st = sort_pool.tile([128, _NSLOT * 2 * BLK], BF16, name="st")
                    pbufq = pbuf.rearrange("p (qq lf) -> p qq lf", qq=2)

                    def ap_of(pos):
                        kind, idx = pos
                        if kind == "c":
                            return pbufq[:, :, idx * BLK:(idx + 1) * BLK]
                        return st[:, idx * 2 * BLK:(idx + 1) * 2 * BLK] \
                            .rearrange("p (qq f) -> p qq f", qq=2)

                    for (su, sv, du, dv) in _SORT_STEPS:
                        nc.vector.tensor_tensor(
                            ap_of(("s", du)), ap_of(su), ap_of(sv), op=ALU.min)
                        nc.vector.tensor_tensor(
                            ap_of(("s", dv)), ap_of(su), ap_of(sv), op=ALU.max)
                    for r in range(LANES):
                        nc.gpsimd.dma_start(
                            sort_d8[:, ch - 1:ch + 1, r * BLK:(r + 1) * BLK],
                            ap_of(_SORT_FINAL[r]),
                        )


        _emit_phase2(tc, nc, conv_d, sort_d, out, scale128, bias128)


def _emit_stats(nc, fp, stats_sum, stats_sq, cc_in, cc_out,
                gamma, beta, scale128, bias128):
        if True:
            ssum = fp.tile([128, 2], F32)
            nc.vector.reduce_sum(ssum[:, 0:1], stats_sum, axis=mybir.AxisListType.X)
            nc.vector.reduce_sum(ssum[:, 1:2], stats_sq, axis=mybir.AxisListType.X)
            tmp = fp.tile([C, 2], F32)
            nc.sync.dma_start(tmp, ssum[C:2 * C, :])
            comb = fp.tile([C, 2], F32)
            nc.vector.tensor_add(comb, ssum[0:C, :], tmp)
            nc.sync.dma_start(cc_in, comb)
            nc.gpsimd.collective_compute(
                "AllReduce", ALU.add,
                replica_groups=[list(range(NCORES))],
                ins=[cc_in.opt()], outs=[cc_out.opt()],
            )
            gst = fp.tile([C, 2], F32)
            nc.sync.dma_start(gst, cc_out)

            gam = fp.tile([C, 1], F32)
            bet = fp.tile([C, 1], F32)
            nc.sync.dma_start(gam, gamma)
            nc.sync.dma_start(bet, beta)
            mean = fp.tile([C, 1], F32)
            nc.vector.tensor_scalar_mul(mean, gst[:, 0:1], 1.0 / NTOT)
            ex2 = fp.tile([C, 1], F32)
            nc.vector.tensor_scalar_mul(ex2, gst[:, 1:2], 1.0 / NTOT)
            msq = fp.tile([C, 1], F32)
            nc.vector.tensor_mul(msq, mean, mean)
            var = fp.tile([C, 1], F32)
            nc.vector.tensor_sub(var, ex2, msq)
            vpe = fp.tile([C, 1], F32)
            nc.vector.tensor_scalar_add(vpe, var, EPS)
            std = fp.tile([C, 1], F32)
            nc.scalar.sqrt(std, vpe)
            rstd = fp.tile([C, 1], F32)
            nc.vector.reciprocal(rstd, std)
            sc64 = fp.tile([C, 1], F32)
            nc.vector.tensor_mul(sc64, gam, rstd)
            t1v = fp.tile([C, 1], F32)
            nc.vector.tensor_mul(t1v, mean, sc64)
            nb64 = fp.tile([C, 1], F32)
            nc.vector.tensor_sub(nb64, bet, t1v)
            nc.sync.dma_start(scale128[0:C, :], sc64)
            nc.sync.dma_start(scale128[C:2 * C, :], sc64)
            nc.sync.dma_start(bias128[0:C, :], nb64)
            nc.sync.dma_start(bias128[C:2 * C, :], nb64)

def _emit_phase2(tc, nc, conv_d, sort_d, out, scale128, bias128):
        # ------------- phase 2: affine+relu, unpermute, write out -------------
        with tc.tile_pool(name="p2i", bufs=3) as p2i, \
             tc.tile_pool(name="p2o", bufs=3) as p2o:
            for ch in range(NCHUNK):
                for (src_d, c_off) in ((conv_d, 0), (sort_d, C)):
                    pin = p2i.tile([128, CHW], BF16, name="pin")
                    nc.sync.dma_start(pin, src_d[:, ch * CHW:(ch + 1) * CHW])
                    pout = p2o.tile([128, CHW], F32, name="pout")
                    pout5 = pout.rearrange(
                        "p (hb ii wb jj) -> p hb ii wb jj",
                        hb=S, ii=RS, wb=W // RS, jj=RS)
                    for i in range(RS):
                        dst = pout5[:, :, i, :, :].transpose([0, 3, 1, 2])
                        src = pin[:, i * RS * BLK:(i + 1) * RS * BLK]
                        if c_off == 0:
                            # a-stream: relu(scale*x+bias) in one ScalarE op
                            nc.scalar.activation(
                                dst, src, AF.Relu,
                                bias=bias128[:, 0:1], scale=scale128[:, 0:1],
                            )
                        else:
                            # y-stream on VectorE (idle once the sort drains)
                            nc.vector.tensor_scalar(
                                dst, src, scale128[:, 0:1], bias128[:, 0:1],
                                op0=ALU.mult, op1=ALU.add,
                            )
                            nc.vector.tensor_scalar_max(dst, dst, 0.0)
                    for m in range(NB):
                        nc.gpsimd.dma_start(
                            out[m, c_off:c_off + C,
                                RS * S * ch:RS * S * (ch + 1), :],
                            pout[C * m:C * (m + 1), :].rearrange(
                                "p (r q) -> p r q", q=W),
                        )


_PROGRAM = None


def _get_program():
    global _PROGRAM
    if _PROGRAM is not None:
        return _PROGRAM
    nc = bacc.Bacc("TRN2", debug=False, enable_asserts=False,
                   target_bir_lowering=False, num_devices=NCORES)
    x = nc.dram_tensor("x", [NB, C, H, W], BF16, kind="ExternalInput").ap()
    wT = nc.dram_tensor("wT", [C, 9 * C], BF16, kind="ExternalInput").ap()
    gamma = nc.dram_tensor("gamma", [C, 1], F32, kind="ExternalInput").ap()
    beta = nc.dram_tensor("beta", [C, 1], F32, kind="ExternalInput").ap()
    out = nc.dram_tensor("out", [NB, 2 * C, H, W], F32, kind="ExternalOutput").ap()
    with tile.TileContext(nc) as tc:
        _body(tc, nc, x, wT, gamma, beta, out)
    nc.compile()
    _PROGRAM = nc
    return nc


def _in_maps(x, conv_w, gamma, beta):
    bf = ml_dtypes.bfloat16
    xq = np.ascontiguousarray(np.asarray(x, np.float32)).astype(bf)
    wTm = np.ascontiguousarray(
        np.asarray(conv_w, np.float32).transpose(1, 2, 3, 0)
    ).reshape(C, 9 * C).astype(bf)
    g = np.ascontiguousarray(np.asarray(gamma, np.float32).reshape(C, 1))
    b = np.ascontiguousarray(np.asarray(beta, np.float32).reshape(C, 1))
    return [
        {"x": xq[NB * k:NB * (k + 1)], "wT": wTm, "gamma": g, "beta": b}
        for k in range(NCORES)
    ]


def run(x, conv_w, conv_b, gamma, beta, **spmd_kwargs):
    nc = _get_program()
    res = run_bass_kernel_spmd(
        nc, _in_maps(x, conv_w, gamma, beta),
        core_ids=list(range(NCORES)), **spmd_kwargs)
    full = np.concatenate(
        [res.results[k]["out"] for k in range(NCORES)], axis=0)
    return full, res


def kernel(x, conv_w, conv_b, gamma, beta):
    full, _ = run(x, conv_w, conv_b, gamma, beta)
    return full
